# revision 45
# baseline (speedup 1.0000x reference)
"""Llama GQA attention (B=2, T=2048, D=2048, 32 heads / 8 KV heads, hd=64) on
8 Trainium2 NeuronCores.

Strategy: tensor-parallel over heads. Each core owns 4 q-heads + 1 kv-head:
wq/wk/wv output-dim sharded, wo input-dim sharded; each core emits a partial
[4096, 2048] o-proj output and the host sums the 8 partials.

Device-side layout tricks:
  - Fully fused single-phase pipeline: projections, RoPE, attention and
    o-proj live in one tile-pool scope so the Tile scheduler can interleave
    them freely.  Projection matmuls for the sub-tile needed two attention
    tiles later act as PE filler inside scalar-bound attention stretches;
    the previous tile's o-proj fills the rest.  This keeps the PE busy
    end-to-end (the PE p-state drops to 1.2 GHz after any idle gap and needs
    3us of continuous work to re-reach 2.4 GHz).
  - x is shipped pre-transposed (xt [2048, 4096]); q/k/v projections run as
    out[d, t] = w.T @ xt with no on-device transposes of x.
  - RoPE pair de-interleave is folded into a host-side permutation of the
    wq/wk columns, arranged so the re/im partner sits 16 partitions away:
    the on-device partner fetch is a single DVE stream_shuffle (no DMA) and
    the rotation itself is all-bf16 SBUF math on the DVE.
  - v is transposed to natural layout with f32 PE transposes packed 4-to-a-
    PSUM-tile, drained by one strided scalar copy.
  - Softmax runs on transposed scores (scoresT[k, q]); the denominator comes
    free from a ones-column appended to v; no max-subtraction is needed
    (inputs are scaled so exp cannot overflow).
  - Causality at block granularity; within diagonal-band k-block pairs the
    second block's live columns are packed immediately after the first's so
    a single Exp covers exactly the causal area (no wasted ACT columns).
  - PSUM budget (8 banks): scores 2x[128,1024] (4), AV accum 2x[128,512]
    (2), o-proj/projection shared 2x[128,512] (2).
  - All DMAs ride the SP (sync) queue: DMA triggers on the Activation queue
    hit fabric flow-control waits that block the exps behind them.  Bulk x
    is emitted in small rate-limited batches between compute groups so no
    flow-control wait ever sits ahead of latency-critical transfers, and
    constants are interleaved into the first x group by need time.
  - The exp-light (j=0) tile runs last so the tail (final o-proj + reserved
    o-proj slice) stays PE-bound instead of draining behind the exp stream.
"""
import sys

for _p in ("/opt/trn_rl_repo", "/root/.axon_site", "/root/.axon_site/_ro/trn_rl_repo",
           "/root/.axon_site/_ro/pypackages"):
    if _p not in sys.path:
        sys.path.append(_p)

import numpy as np
import ml_dtypes

import concourse.bass as bass
import concourse.mybir as mybir
import concourse.tile as tile
from concourse import bacc
from concourse.bass_utils import run_bass_kernel_spmd

f32 = mybir.dt.float32
bf16 = mybir.dt.bfloat16
AF = mybir.ActivationFunctionType

B, T, D = 2, 2048, 2048
H, HKV, HD = 32, 8, 64
NCORES = 8
HPC = H // NCORES            # q heads per core (4)
DQC = HPC * HD               # 256 q channels per core
N = B * T                    # 4096 flattened tokens
KC = D // 128                # 16 contraction chunks for projections
NT = N // 512                # 8 token sub-tiles of 512 for projections
QT = T // 512                # 4 q-tiles of 512 per batch
KB = T // 128                # 16 k-blocks of 128 per batch
ROPE_THETA = 10000.0

_nc_cache = [None]


def build():
    if _nc_cache[0] is not None:
        return _nc_cache[0]
    nc = bacc.Bacc()
    xt = nc.declare_dram_parameter("xt", [D, N], bf16, isOutput=False)
    wqkv = nc.declare_dram_parameter("wqkv", [D, DQC + 2 * HD], bf16, isOutput=False)
    wo = nc.declare_dram_parameter("wo", [DQC, D], bf16, isOutput=False)
    cs = nc.declare_dram_parameter("cs", [2, 128, N], bf16, isOutput=False)
    tri = nc.declare_dram_parameter("tri", [128, 128], bf16, isOutput=False)
    ident = nc.declare_dram_parameter("ident", [64, 64], f32, isOutput=False)
    ones = nc.declare_dram_parameter("ones", [128, 32], bf16, isOutput=False)
    out = nc.declare_dram_parameter("out", [N, D], bf16, isOutput=True)
    DEBUG = bool(__import__("os").environ.get("KDBG"))
    if DEBUG:
        dbg_q01 = nc.declare_dram_parameter("dbg_q01", [128, N], bf16, isOutput=True)
        dbg_q23 = nc.declare_dram_parameter("dbg_q23", [128, N], bf16, isOutput=True)
        dbg_kk = nc.declare_dram_parameter("dbg_kk", [128, N], bf16, isOutput=True)
        dbg_vnat = nc.declare_dram_parameter("dbg_vnat", [128, (N // 128) * 65], bf16,
                                             isOutput=True)

    with tile.TileContext(nc) as tc:
        with tc.tile_pool(name="pers", bufs=1) as pers, \
             tc.tile_pool(name="sb", bufs=1) as sb, \
             tc.tile_pool(name="ps", bufs=1, space="PSUM") as ps:
            wqkv_sb = pers.tile([128, KC, 384], bf16)
            wo0 = pers.tile([128, D], bf16)
            wo1 = pers.tile([128, D], bf16)
            tri_sb = pers.tile([128, 128], bf16)
            ident_f = pers.tile([64, 64], f32)
            q01 = pers.tile([128, N], bf16)      # heads 0,1 qT
            q23 = pers.tile([128, N], bf16)      # heads 2,3 qT
            kk = pers.tile([128, N], bf16)       # kT duplicated into both halves
            vnat = pers.tile([128, N // 128, 65], bf16)  # v natural + ones col
            cos_f = pers.tile([128, N], bf16)
            sin_f = pers.tile([128, N], bf16)

            # ---- input DMAs ----
            # sync (SP) queue: weights interleaved with group-0 x chunks so the
            # first projection matmul can start as soon as chunk 0 lands.
            # Everything rides the sync (SP) queue: keeping the Scalar engine
            # queue free of DMA triggers matters — fabric flow-control waits
            # there would block the exps/copies queued behind them.
            # Sub-0's x chunks load first as [128,512] slices: attention tile
            # (0,0) only needs sub 0, so its projection completes ~10us sooner
            # than waiting for a full [128,1024] group-0 stream.
            xg = [[None] * KC for _ in range(4)]
            xs = [[None] * KC for _ in range(2)]
            for c in range(KC):
                nc.sync.dma_start(out=wqkv_sb[:, c, :],
                                  in_=wqkv[c * 128:(c + 1) * 128, :])
                t_ = sb.tile([128, 512], bf16, tag="x0", bufs=32, name=f"xs0_{c}")
                nc.sync.dma_start(out=t_, in_=xt[c * 128:(c + 1) * 128, 0:512])
                xs[0][c] = t_
                if c == 2:
                    nc.sync.dma_start(out=tri_sb, in_=tri[:])
                    nc.sync.dma_start(out=ident_f, in_=ident[:])
                    nc.sync.dma_start(out=vnat[:, :, 64:65],
                                      in_=ones[:].unsqueeze(2))
            nc.sync.dma_start(out=cos_f[:, 0:2048], in_=cs[0, :, 0:2048])
            nc.sync.dma_start(out=sin_f[:, 0:2048], in_=cs[1, :, 0:2048])
            for c in range(KC):
                t_ = sb.tile([128, 512], bf16, tag="x0", bufs=32, name=f"xs1_{c}")
                nc.sync.dma_start(out=t_, in_=xt[c * 128:(c + 1) * 128, 512:1024])
                xs[1][c] = t_

            # Remaining bulk (x groups 1-3, wo) rides the sync queue in small
            # rate-limited batches sprinkled through the compute stream, so no
            # flow-control wait ever sits ahead of latency-critical work.
            bulk = []
            for g in range(1, 4):
                for c in range(KC):
                    t_ = sb.tile([128, 1024], bf16, tag="xg", bufs=33,
                                 name=f"xg{g}_{c}")
                    xg[g][c] = t_
                    bulk.append((t_, xt[c * 128:(c + 1) * 128,
                                        g * 1024:(g + 1) * 1024]))
                if g == 1:
                    bulk.append((wo0, wo[0:128, :]))
                    bulk.append((wo1, wo[128:256, :]))
                    bulk.append((cos_f[:, 2048:], cs[0, :, 2048:]))
                    bulk.append((sin_f[:, 2048:], cs[1, :, 2048:]))

            def emit_bulk(k):
                for _ in range(k):
                    if bulk:
                        dst, src = bulk.pop(0)
                        nc.sync.dma_start(out=dst, in_=src)

            # ---- projection + RoPE helpers ----
            def proj_q2(n):
                # both q head-pairs chunk-major: consumption tracks x-chunk
                # arrival so the PE queue never stalls deep inside a burst
                g, s0 = n // 2, (n % 2) * 512
                pqa = ps.tile([128, 512], f32, tag="op", bufs=2, name=f"pq0_{n}")
                pqb = ps.tile([128, 512], f32, tag="op", bufs=2, name=f"pq1_{n}")
                for c in range(KC):
                    st, sp = (c == 0), (c == KC - 1)
                    nc.tensor.matmul(pqa, wqkv_sb[:, c, 0:128],
                                     xg[g][c][:, s0:s0 + 512], start=st, stop=sp)
                    nc.tensor.matmul(pqb, wqkv_sb[:, c, 128:256],
                                     xg[g][c][:, s0:s0 + 512], start=st, stop=sp)
                qca = sb.tile([128, 512], bf16, tag="qc", bufs=3, name="qc")
                nc.scalar.copy(qca, pqa)
                qcb = sb.tile([128, 512], bf16, tag="qc", bufs=3, name="qc")
                nc.scalar.copy(qcb, pqb)
                return qca, qcb

            # rope pair partner: a +-16 partition shift within each 32-quadrant
            # (host channel layout arranged so this is one DVE stream_shuffle)
            SWAP16 = list(range(16, 32)) + list(range(16))

            def rope_q(n, part, qc):
                t0 = n * 512
                qbuf = q01 if part == 0 else q23
                qs = sb.tile([128, 512], bf16, tag="qs", bufs=3, name="qs")
                nc.vector.stream_shuffle(qs, qc, SWAP16)
                t1 = sb.tile([128, 512], bf16, tag="t1", bufs=3, name="t1")
                t2 = sb.tile([128, 512], bf16, tag="t2", bufs=3, name="t2")
                nc.vector.tensor_mul(t1, qc, cos_f[:, t0:t0 + 512])
                nc.vector.tensor_mul(t2, qs, sin_f[:, t0:t0 + 512])
                nc.vector.tensor_add(qbuf[:, t0:t0 + 512], t1, t2)

            def proj_kv(n, tag="op"):
                g, s0 = n // 2, (n % 2) * 512
                pkv = ps.tile([128, 512], f32, tag=tag, bufs=2, name=f"pkv_{n}")
                for c in range(KC):
                    nc.tensor.matmul(pkv, wqkv_sb[:, c, 256:384],
                                     xg[g][c][:, s0:s0 + 512],
                                     start=(c == 0), stop=(c == KC - 1))
                kc_ = sb.tile([64, 512], bf16, tag="qc", bufs=3, name="kc")
                nc.scalar.copy(kc_, pkv[0:64, :])
                vts = sb.tile([64, 512], f32, tag="vts", bufs=2, name="vts")
                nc.scalar.copy(vts, pkv[64:128, :])
                return kc_, vts

            def rope_kv(n, kc_, vts):
                t0 = n * 512
                ks = sb.tile([64, 512], bf16, tag="qs", bufs=3, name="ks")
                nc.vector.stream_shuffle(ks, kc_, SWAP16)
                t1k = sb.tile([64, 512], bf16, tag="t1", bufs=3, name="t1k")
                t2k = sb.tile([64, 512], bf16, tag="t2", bufs=3, name="t2k")
                nc.vector.tensor_mul(t1k, kc_, cos_f[0:64, t0:t0 + 512])
                nc.vector.tensor_mul(t2k, ks, sin_f[0:64, t0:t0 + 512])
                nc.vector.tensor_add(kk[0:64, t0:t0 + 512], t1k, t2k)
                nc.sync.dma_start(out=kk[64:128, t0:t0 + 512],
                                  in_=kk[0:64, t0:t0 + 512])
                # v natural layout: 4 PE transposes packed into one op-tag
                # PSUM tile, drained by a single strided scalar copy.
                trp = ps.tile([128, 512], f32, tag="op", bufs=2, name="trp")
                for s4 in range(4):
                    nc.tensor.transpose(trp[:, s4 * 64:(s4 + 1) * 64],
                                        vts[:, s4 * 128:(s4 + 1) * 128], ident_f)
                nc.scalar.copy(
                    vnat[:, n * 4:n * 4 + 4, 0:64],
                    trp[:, 0:256].rearrange("p (b v) -> p b v", b=4))

            def oproj_slice(pend, st, cast_scalar=False):
                # one quarter (token sub-block st) of a tile's o-proj
                poT0, poT1, pq0 = pend
                tk = pq0 + st * 128
                for dn2 in range(2):
                    osb = sb.tile([128, 1024], bf16, tag="osb", bufs=3, name="osb")
                    for kq in range(2):
                        dn = dn2 * 2 + kq
                        pop = ps.tile([128, 512], f32, tag="op", bufs=2, name="pop")
                        nc.tensor.matmul(pop, poT0[:, st * 128:(st + 1) * 128],
                                         wo0[:, dn * 512:(dn + 1) * 512],
                                         start=True, stop=False)
                        nc.tensor.matmul(pop, poT1[:, st * 128:(st + 1) * 128],
                                         wo1[:, dn * 512:(dn + 1) * 512],
                                         start=False, stop=True)
                        if cast_scalar and kq == 1:
                            # exp-light stretches: drain half the pops on the
                            # Scalar engine so DVE isn't the serial bottleneck
                            nc.scalar.copy(osb[:, kq * 512:(kq + 1) * 512], pop)
                        else:
                            nc.vector.tensor_copy(osb[:, kq * 512:(kq + 1) * 512],
                                                  pop)
                    nc.sync.dma_start(out=out[tk:tk + 128, dn2 * 1024:(dn2 + 1) * 1024],
                                      in_=osb)

            # ---- prologue: sub-tiles 0 and 1, chunk-major across 6 PSUM
            # accumulators so ~6 matmuls become ready per arriving x chunk
            # (the PE would otherwise sit half idle during the g0 stream).
            pq01_ = [ps.tile([128, 512], f32, tag="op", bufs=2, name=f"pq0_{n}")
                     for n in range(2)]
            pq23_ = [ps.tile([128, 1024], f32, tag="sc", bufs=2, name=f"pq1_{n}")
                     for n in range(2)]
            pkv_ = [ps.tile([128, 512], f32, tag="av", bufs=2, name=f"pkv_{n}")
                    for n in range(2)]
            for c in range(KC):
                st, sp = (c == 0), (c == KC - 1)
                for n in range(2):
                    nc.tensor.matmul(pq01_[n], wqkv_sb[:, c, 0:128],
                                     xs[n][c], start=st, stop=sp)
                    nc.tensor.matmul(pq23_[n][:, 0:512], wqkv_sb[:, c, 128:256],
                                     xs[n][c], start=st, stop=sp)
                    nc.tensor.matmul(pkv_[n], wqkv_sb[:, c, 256:384],
                                     xs[n][c], start=st, stop=sp)
            for n in range(2):
                qc = sb.tile([128, 512], bf16, tag="qc", bufs=3, name="qc")
                nc.scalar.copy(qc, pq01_[n])
                rope_q(n, 0, qc)
                emit_bulk(6)
                qc = sb.tile([128, 512], bf16, tag="qc", bufs=3, name="qc")
                nc.scalar.copy(qc, pq23_[n][:, 0:512])
                rope_q(n, 1, qc)
                emit_bulk(6)
                kc_ = sb.tile([64, 512], bf16, tag="qc", bufs=3, name="kc")
                nc.scalar.copy(kc_, pkv_[n][0:64, :])
                vts = sb.tile([64, 512], f32, tag="vts", bufs=2, name="vts")
                nc.scalar.copy(vts, pkv_[n][64:128, :])
                rope_kv(n, kc_, vts)
                emit_bulk(6)

            # ---- fused attention + o-proj + interleaved projections ----
            # tile (b, j) plus the projection sub-tile needed later; the final
            # tile is exp-light (j=0) so the tail stays PE-bound
            sched = [(0, 0, 2), (0, 1, 3), (0, 2, 4), (0, 3, 5),
                     (1, 1, 6), (1, 2, 7), (1, 3, None), (1, 0, None)]
            pending = None
            for b, j, fsub in sched:
                q0 = b * T + j * 512
                oT0 = sb.tile([128, 512], bf16, tag="oT0", bufs=2)
                oT1 = sb.tile([128, 512], bf16, tag="oT1", bufs=2)
                fq = None
                for h in range(HPC):
                    qbuf = q01 if h < 2 else q23
                    base = (h % 2) * 64
                    oT = oT0 if h < 2 else oT1
                    pav = ps.tile([128, 512], f32, tag="av", bufs=2, name="pav")
                    nkb = 4 * j + 4
                    for pr in range(nkb // 2):
                        psc = ps.tile([128, 1024], f32, tag="sc", bufs=2, name="psc")
                        es = sb.tile([128, 1024], bf16, tag="es", bufs=3, name="es")
                        kb0, kb1 = 2 * pr, 2 * pr + 1
                        di0, di1 = kb0 - 4 * j, kb1 - 4 * j
                        c00 = 128 * di0 if di0 > 0 else 0
                        c01 = 128 * di1 if di1 > 0 else 0
                        hi = 1024 - c01
                        k0 = b * T + kb0 * 128
                        k1 = b * T + kb1 * 128
                        nc.tensor.matmul(
                            psc[:, c00:512],
                            kk[base:base + 64, k0:k0 + 128],
                            qbuf[base:base + 64, q0 + c00:q0 + 512],
                            start=True, stop=True)
                        nc.tensor.matmul(
                            psc[:, 512:hi],
                            kk[base:base + 64, k1:k1 + 128],
                            qbuf[base:base + 64, q0 + c01:q0 + 512],
                            start=True, stop=True)
                        nc.scalar.activation(es[:, c00:hi], psc[:, c00:hi],
                                             AF.Exp, scale=0.125)
                        if di0 >= 0:
                            nc.vector.tensor_mul(es[:, c00:c00 + 128],
                                                 es[:, c00:c00 + 128], tri_sb)
                            nc.vector.tensor_mul(es[:, 512:640],
                                                 es[:, 512:640], tri_sb)
                        nc.tensor.matmul(
                            pav[0:65, c00:512],
                            vnat[:, b * KB + kb0, :],
                            es[:, c00:512],
                            start=(kb0 == 0), stop=False)
                        nc.tensor.matmul(
                            pav[0:65, c01:512],
                            vnat[:, b * KB + kb1, :],
                            es[:, 512:hi],
                            start=False, stop=(kb1 == nkb - 1))
                    rrow = sb.tile([1, 512], f32, tag="rr", bufs=2, name="rr")
                    nc.vector.tensor_copy(rrow, pav[64:65, :])
                    rec = sb.tile([1, 512], f32, tag="rec", bufs=2, name="rec")
                    nc.vector.reciprocal_approx_fast(out=rec, in_=rrow)
                    rb = sb.tile([64, 512], f32, tag="rb", bufs=2, name="rb")
                    nc.gpsimd.partition_broadcast(rb, rec)
                    nc.vector.tensor_mul(oT[base:base + 64, :], pav[0:64, :], rb)
                    if pending is not None and not ((b, j) == sched[-1][:2]
                                                    and h == 3):
                        # (reserve the very last pending slice: it covers the
                        # final head's normalize-chain latency after the loop)
                        oproj_slice(pending, h, cast_scalar=(j <= 1))
                    emit_bulk(2)
                    if fsub is not None:
                        if h == 0:
                            fq = proj_q2(fsub)
                        elif h == 1:
                            rope_q(fsub, 0, fq[0])
                            rope_q(fsub, 1, fq[1])
                        elif h == 2:
                            fq = proj_kv(fsub)
                        else:
                            rope_kv(fsub, *fq)
                if (b, j) == sched[-1][:2]:
                    oproj_slice(pending, 3, cast_scalar=True)
                pending = (oT0, oT1, q0)
            for st in range(4):
                oproj_slice(pending, st, cast_scalar=True)
            if DEBUG:
                nc.sync.dma_start(out=dbg_q01[:], in_=q01[:])
                nc.sync.dma_start(out=dbg_q23[:], in_=q23[:])
                nc.sync.dma_start(out=dbg_kk[:], in_=kk[:])
                nc.sync.dma_start(
                    out=dbg_vnat[:].rearrange("p (b c) -> p b c", b=N // 128),
                    in_=vnat[:])

    nc.compile()
    _nc_cache[0] = nc
    return nc


def prep_inputs(x, wq, wk, wv, wo):
    x = np.asarray(x, np.float32)
    wq = np.asarray(wq, np.float32)
    wk = np.asarray(wk, np.float32)
    wv = np.asarray(wv, np.float32)
    wo = np.asarray(wo, np.float32)

    xt = np.ascontiguousarray(x.reshape(N, D).T.astype(ml_dtypes.bfloat16))  # [D, N]

    # de-interleave RoPE pairs inside each head's 64 columns: re/im partners
    # land 16 partitions apart so the on-device swap is one stream_shuffle
    # (which can only permute within 32-partition quadrants)
    deint = np.concatenate([
        np.arange(0, 32, 2), np.arange(1, 32, 2),      # re_0..15, im_0..15
        np.arange(32, 64, 2), np.arange(33, 64, 2),    # re_16..31, im_16..31
    ])
    qperm = (np.arange(H)[:, None] * HD + deint[None, :]).reshape(-1)
    kperm = (np.arange(HKV)[:, None] * HD + deint[None, :]).reshape(-1)
    wq_p = wq[:, qperm]
    wk_p = wk[:, kperm]

    # rope tables in the matching row order
    inv = 1.0 / (ROPE_THETA ** (np.arange(0, HD, 2, dtype=np.float64) / HD))
    tpos = np.arange(T, dtype=np.float64)
    ang = np.outer(tpos, inv)                                        # [T, 32]
    cosv = np.cos(ang).astype(np.float32).T                          # [32, T]
    sinv = np.sin(ang).astype(np.float32).T
    fidx = np.concatenate([np.arange(16), np.arange(16),
                           np.arange(16, 32), np.arange(16, 32)])
    sgn = np.concatenate([-np.ones(16), np.ones(16),
                          -np.ones(16), np.ones(16)]).astype(np.float32)
    cos_half = cosv[fidx]                                            # [64, T]
    sin_half = sinv[fidx] * sgn[:, None]
    cs = np.stack([
        np.tile(np.tile(cos_half, (2, 1)), (1, B)),
        np.tile(np.tile(sin_half, (2, 1)), (1, B)),
    ]).astype(ml_dtypes.bfloat16)                                    # [2, 128, N]

    p = np.arange(128)[:, None]
    c = np.arange(128)[None, :]
    tri = (p <= c).astype(ml_dtypes.bfloat16)                        # [128, 128]

    ones = np.ones((128, 32), ml_dtypes.bfloat16)
    ident = np.eye(64, dtype=np.float32)

    in_maps = []
    for core in range(NCORES):
        wq_c = wq_p[:, core * DQC:(core + 1) * DQC]
        wk_c = wk_p[:, core * HD:(core + 1) * HD]
        wv_c = wv[:, core * HD:(core + 1) * HD]
        wqkv = np.ascontiguousarray(
            np.concatenate([wq_c, wk_c, wv_c], axis=1).astype(ml_dtypes.bfloat16))
        wo_c = np.ascontiguousarray(
            wo[core * DQC:(core + 1) * DQC, :].astype(ml_dtypes.bfloat16))
        in_maps.append({
            "xt": xt, "wqkv": wqkv, "wo": wo_c, "cs": cs,
            "tri": tri, "ones": ones, "ident": ident,
        })
    return in_maps


def kernel(x, wq, wk, wv, wo):
    nc = build()
    in_maps = prep_inputs(x, wq, wk, wv, wo)
    res = run_bass_kernel_spmd(nc, in_maps, list(range(NCORES)))
    acc = np.zeros((N, D), np.float64)
    for core in range(NCORES):
        acc += res.results[core]["out"].astype(np.float32)
    return acc.astype(np.float32).reshape(B, T, D)


# revision 48
# speedup vs baseline: 1.0160x; 1.0160x over previous
"""Llama GQA attention (B=2, T=2048, D=2048, 32 heads / 8 KV heads, hd=64) on
8 Trainium2 NeuronCores.

Strategy: tensor-parallel over heads. Each core owns 4 q-heads + 1 kv-head:
wq/wk/wv output-dim sharded, wo input-dim sharded; each core emits a partial
[4096, 2048] o-proj output and the host sums the 8 partials.

Device-side layout tricks:
  - Fully fused single-phase pipeline: projections, RoPE, attention and
    o-proj live in one tile-pool scope so the Tile scheduler can interleave
    them freely.  Projection matmuls for the sub-tile needed two attention
    tiles later act as PE filler inside scalar-bound attention stretches;
    the previous tile's o-proj fills the rest.  This keeps the PE busy
    end-to-end (the PE p-state drops to 1.2 GHz after any idle gap and needs
    3us of continuous work to re-reach 2.4 GHz).
  - x is shipped pre-transposed (xt [2048, 4096]); q/k/v projections run as
    out[d, t] = w.T @ xt with no on-device transposes of x.
  - RoPE pair de-interleave is folded into a host-side permutation of the
    wq/wk columns, arranged so the re/im partner sits 16 partitions away:
    the on-device partner fetch is a single DVE stream_shuffle (no DMA) and
    the rotation itself is all-bf16 SBUF math on the DVE.
  - v is transposed to natural layout with f32 PE transposes packed 4-to-a-
    PSUM-tile, drained by one strided scalar copy.
  - Softmax runs on transposed scores (scoresT[k, q]); the denominator comes
    free from a ones-column appended to v; no max-subtraction is needed
    (inputs are scaled so exp cannot overflow).
  - Causality at block granularity; within diagonal-band k-block pairs the
    second block's live columns are packed immediately after the first's so
    a single Exp covers exactly the causal area (no wasted ACT columns).
  - PSUM budget (8 banks): scores 2x[128,1024] (4), AV accum 2x[128,512]
    (2), o-proj/projection shared 2x[128,512] (2).
  - All DMAs ride the SP (sync) queue: DMA triggers on the Activation queue
    hit fabric flow-control waits that block the exps behind them.  Bulk x
    is emitted in small rate-limited batches between compute groups so no
    flow-control wait ever sits ahead of latency-critical transfers, and
    constants are interleaved into the first x group by need time.
  - The exp-light (j=0) tile runs last so the tail (final o-proj + reserved
    o-proj slice) stays PE-bound instead of draining behind the exp stream.
"""
import sys

for _p in ("/opt/trn_rl_repo", "/root/.axon_site", "/root/.axon_site/_ro/trn_rl_repo",
           "/root/.axon_site/_ro/pypackages"):
    if _p not in sys.path:
        sys.path.append(_p)

import numpy as np
import ml_dtypes

import concourse.bass as bass
import concourse.mybir as mybir
import concourse.tile as tile
from concourse import bacc
from concourse.bass_utils import run_bass_kernel_spmd

f32 = mybir.dt.float32
bf16 = mybir.dt.bfloat16
AF = mybir.ActivationFunctionType

B, T, D = 2, 2048, 2048
H, HKV, HD = 32, 8, 64
NCORES = 8
HPC = H // NCORES            # q heads per core (4)
DQC = HPC * HD               # 256 q channels per core
N = B * T                    # 4096 flattened tokens
KC = D // 128                # 16 contraction chunks for projections
NT = N // 512                # 8 token sub-tiles of 512 for projections
QT = T // 512                # 4 q-tiles of 512 per batch
KB = T // 128                # 16 k-blocks of 128 per batch
ROPE_THETA = 10000.0

_nc_cache = [None]


def build():
    if _nc_cache[0] is not None:
        return _nc_cache[0]
    nc = bacc.Bacc()
    xt = nc.declare_dram_parameter("xt", [D, N], bf16, isOutput=False)
    wqkv = nc.declare_dram_parameter("wqkv", [D, DQC + 2 * HD], bf16, isOutput=False)
    wo = nc.declare_dram_parameter("wo", [DQC, D], bf16, isOutput=False)
    cs = nc.declare_dram_parameter("cs", [2, 128, N], bf16, isOutput=False)
    tri = nc.declare_dram_parameter("tri", [128, 128], bf16, isOutput=False)
    ident = nc.declare_dram_parameter("ident", [64, 64], f32, isOutput=False)
    ones = nc.declare_dram_parameter("ones", [128, 32], bf16, isOutput=False)
    out = nc.declare_dram_parameter("out", [N, D], bf16, isOutput=True)
    DEBUG = bool(__import__("os").environ.get("KDBG"))
    if DEBUG:
        dbg_q01 = nc.declare_dram_parameter("dbg_q01", [128, N], bf16, isOutput=True)
        dbg_q23 = nc.declare_dram_parameter("dbg_q23", [128, N], bf16, isOutput=True)
        dbg_kk = nc.declare_dram_parameter("dbg_kk", [128, N], bf16, isOutput=True)
        dbg_vnat = nc.declare_dram_parameter("dbg_vnat", [128, (N // 128) * 65], bf16,
                                             isOutput=True)

    with tile.TileContext(nc) as tc:
        with tc.tile_pool(name="pers", bufs=1) as pers, \
             tc.tile_pool(name="sb", bufs=1) as sb, \
             tc.tile_pool(name="ps", bufs=1, space="PSUM") as ps:
            wqkv_sb = pers.tile([128, KC, 384], bf16)
            wo0 = pers.tile([128, D], bf16)
            wo1 = pers.tile([128, D], bf16)
            tri_sb = pers.tile([128, 128], bf16)
            ident_f = pers.tile([64, 64], f32)
            q01 = pers.tile([128, N], bf16)      # heads 0,1 qT
            q23 = pers.tile([128, N], bf16)      # heads 2,3 qT
            kk = pers.tile([128, N], bf16)       # kT duplicated into both halves
            vnat = pers.tile([128, N // 128, 65], bf16)  # v natural + ones col
            cos_f = pers.tile([128, N], bf16)
            sin_f = pers.tile([128, N], bf16)

            # ---- input DMAs ----
            # sync (SP) queue: weights interleaved with group-0 x chunks so the
            # first projection matmul can start as soon as chunk 0 lands.
            # Everything rides the sync (SP) queue: keeping the Scalar engine
            # queue free of DMA triggers matters — fabric flow-control waits
            # there would block the exps/copies queued behind them.
            # Constants are interleaved into the wqkv+g0 stream by need time.
            xg = [[None] * KC for _ in range(4)]
            for c in range(KC):
                nc.sync.dma_start(out=wqkv_sb[:, c, :],
                                  in_=wqkv[c * 128:(c + 1) * 128, :])
                t_ = sb.tile([128, 1024], bf16, tag="xg", bufs=48, name=f"xg0_{c}")
                nc.sync.dma_start(out=t_, in_=xt[c * 128:(c + 1) * 128, 0:1024])
                xg[0][c] = t_
                if c == 2:
                    nc.sync.dma_start(out=tri_sb, in_=tri[:])
                    nc.sync.dma_start(out=ident_f, in_=ident[:])
                    nc.sync.dma_start(out=vnat[:, :, 64:65],
                                      in_=ones[:].unsqueeze(2))
                elif c == 7:
                    nc.sync.dma_start(out=cos_f, in_=cs[0])
                elif c == 8:
                    nc.sync.dma_start(out=sin_f, in_=cs[1])

            # Remaining bulk (x groups 1-3, wo) rides the sync queue in small
            # rate-limited batches sprinkled through the compute stream, so no
            # flow-control wait ever sits ahead of latency-critical work.
            bulk = []
            for g in range(1, 4):
                for c in range(KC):
                    t_ = sb.tile([128, 1024], bf16, tag="xg", bufs=48,
                                 name=f"xg{g}_{c}")
                    xg[g][c] = t_
                    bulk.append((t_, xt[c * 128:(c + 1) * 128,
                                        g * 1024:(g + 1) * 1024]))
                if g == 1:
                    bulk.append((wo0, wo[0:128, :]))
                    bulk.append((wo1, wo[128:256, :]))

            def emit_bulk(k):
                for _ in range(k):
                    if bulk:
                        dst, src = bulk.pop(0)
                        nc.sync.dma_start(out=dst, in_=src)

            # ---- projection + RoPE helpers ----
            def proj_q2(n):
                # both q head-pairs chunk-major: consumption tracks x-chunk
                # arrival so the PE queue never stalls deep inside a burst
                g, s0 = n // 2, (n % 2) * 512
                pqa = ps.tile([128, 512], f32, tag="op", bufs=2, name=f"pq0_{n}")
                pqb = ps.tile([128, 512], f32, tag="op", bufs=2, name=f"pq1_{n}")
                for c in range(KC):
                    st, sp = (c == 0), (c == KC - 1)
                    nc.tensor.matmul(pqa, wqkv_sb[:, c, 0:128],
                                     xg[g][c][:, s0:s0 + 512], start=st, stop=sp)
                    nc.tensor.matmul(pqb, wqkv_sb[:, c, 128:256],
                                     xg[g][c][:, s0:s0 + 512], start=st, stop=sp)
                qca = sb.tile([128, 512], bf16, tag="qc", bufs=3, name="qc")
                nc.scalar.copy(qca, pqa)
                qcb = sb.tile([128, 512], bf16, tag="qc", bufs=3, name="qc")
                nc.scalar.copy(qcb, pqb)
                return qca, qcb

            # rope pair partner: a +-16 partition shift within each 32-quadrant
            # (host channel layout arranged so this is one DVE stream_shuffle)
            SWAP16 = list(range(16, 32)) + list(range(16))

            def rope_q(n, part, qc):
                t0 = n * 512
                qbuf = q01 if part == 0 else q23
                qs = sb.tile([128, 512], bf16, tag="qs", bufs=3, name="qs")
                nc.vector.stream_shuffle(qs, qc, SWAP16)
                t1 = sb.tile([128, 512], bf16, tag="t1", bufs=3, name="t1")
                t2 = sb.tile([128, 512], bf16, tag="t2", bufs=3, name="t2")
                nc.vector.tensor_mul(t1, qc, cos_f[:, t0:t0 + 512])
                nc.vector.tensor_mul(t2, qs, sin_f[:, t0:t0 + 512])
                nc.vector.tensor_add(qbuf[:, t0:t0 + 512], t1, t2)

            def proj_kv(n, tag="op"):
                g, s0 = n // 2, (n % 2) * 512
                pkv = ps.tile([128, 512], f32, tag=tag, bufs=2, name=f"pkv_{n}")
                for c in range(KC):
                    nc.tensor.matmul(pkv, wqkv_sb[:, c, 256:384],
                                     xg[g][c][:, s0:s0 + 512],
                                     start=(c == 0), stop=(c == KC - 1))
                kc_ = sb.tile([64, 512], bf16, tag="qc", bufs=3, name="kc")
                nc.scalar.copy(kc_, pkv[0:64, :])
                vts = sb.tile([64, 512], f32, tag="vts", bufs=2, name="vts")
                nc.scalar.copy(vts, pkv[64:128, :])
                return kc_, vts

            def rope_kv(n, kc_, vts):
                t0 = n * 512
                ks = sb.tile([64, 512], bf16, tag="qs", bufs=3, name="ks")
                nc.vector.stream_shuffle(ks, kc_, SWAP16)
                t1k = sb.tile([64, 512], bf16, tag="t1", bufs=3, name="t1k")
                t2k = sb.tile([64, 512], bf16, tag="t2", bufs=3, name="t2k")
                nc.vector.tensor_mul(t1k, kc_, cos_f[0:64, t0:t0 + 512])
                nc.vector.tensor_mul(t2k, ks, sin_f[0:64, t0:t0 + 512])
                nc.vector.tensor_add(kk[0:64, t0:t0 + 512], t1k, t2k)
                nc.sync.dma_start(out=kk[64:128, t0:t0 + 512],
                                  in_=kk[0:64, t0:t0 + 512])
                # v natural layout: 4 PE transposes packed into one op-tag
                # PSUM tile, drained by a single strided scalar copy.
                trp = ps.tile([128, 512], f32, tag="op", bufs=2, name="trp")
                for s4 in range(4):
                    nc.tensor.transpose(trp[:, s4 * 64:(s4 + 1) * 64],
                                        vts[:, s4 * 128:(s4 + 1) * 128], ident_f)
                nc.scalar.copy(
                    vnat[:, n * 4:n * 4 + 4, 0:64],
                    trp[:, 0:256].rearrange("p (b v) -> p b v", b=4))

            def oproj_slice(pend, st, cast_scalar=False):
                # one quarter (token sub-block st) of a tile's o-proj
                poT0, poT1, pq0 = pend
                tk = pq0 + st * 128
                for dn2 in range(2):
                    osb = sb.tile([128, 1024], bf16, tag="osb", bufs=3, name="osb")
                    for kq in range(2):
                        dn = dn2 * 2 + kq
                        pop = ps.tile([128, 512], f32, tag="op", bufs=2, name="pop")
                        nc.tensor.matmul(pop, poT0[:, st * 128:(st + 1) * 128],
                                         wo0[:, dn * 512:(dn + 1) * 512],
                                         start=True, stop=False)
                        nc.tensor.matmul(pop, poT1[:, st * 128:(st + 1) * 128],
                                         wo1[:, dn * 512:(dn + 1) * 512],
                                         start=False, stop=True)
                        if cast_scalar and kq == 1:
                            # exp-light stretches: drain half the pops on the
                            # Scalar engine so DVE isn't the serial bottleneck
                            nc.scalar.copy(osb[:, kq * 512:(kq + 1) * 512], pop)
                        else:
                            nc.vector.tensor_copy(osb[:, kq * 512:(kq + 1) * 512],
                                                  pop)
                    nc.sync.dma_start(out=out[tk:tk + 128, dn2 * 1024:(dn2 + 1) * 1024],
                                      in_=osb)

            # ---- prologue: sub-tiles 0 and 1, chunk-major across 6 PSUM
            # accumulators so ~6 matmuls become ready per arriving x chunk
            # (the PE would otherwise sit half idle during the g0 stream).
            pq01_ = [ps.tile([128, 512], f32, tag="op", bufs=2, name=f"pq0_{n}")
                     for n in range(2)]
            pq23_ = [ps.tile([128, 1024], f32, tag="sc", bufs=2, name=f"pq1_{n}")
                     for n in range(2)]
            pkv_ = [ps.tile([128, 512], f32, tag="av", bufs=2, name=f"pkv_{n}")
                    for n in range(2)]
            for c in range(KC):
                st, sp = (c == 0), (c == KC - 1)
                for n in range(2):
                    s0 = n * 512
                    nc.tensor.matmul(pq01_[n], wqkv_sb[:, c, 0:128],
                                     xg[0][c][:, s0:s0 + 512], start=st, stop=sp)
                    nc.tensor.matmul(pq23_[n][:, 0:512], wqkv_sb[:, c, 128:256],
                                     xg[0][c][:, s0:s0 + 512], start=st, stop=sp)
                    nc.tensor.matmul(pkv_[n], wqkv_sb[:, c, 256:384],
                                     xg[0][c][:, s0:s0 + 512], start=st, stop=sp)
            for n in range(2):
                qc = sb.tile([128, 512], bf16, tag="qc", bufs=3, name="qc")
                nc.scalar.copy(qc, pq01_[n])
                rope_q(n, 0, qc)
                emit_bulk(6)
                qc = sb.tile([128, 512], bf16, tag="qc", bufs=3, name="qc")
                nc.scalar.copy(qc, pq23_[n][:, 0:512])
                rope_q(n, 1, qc)
                emit_bulk(6)
                kc_ = sb.tile([64, 512], bf16, tag="qc", bufs=3, name="kc")
                nc.scalar.copy(kc_, pkv_[n][0:64, :])
                vts = sb.tile([64, 512], f32, tag="vts", bufs=2, name="vts")
                nc.scalar.copy(vts, pkv_[n][64:128, :])
                rope_kv(n, kc_, vts)
                emit_bulk(6)

            # ---- fused attention + o-proj + interleaved projections ----
            # tile (b, j) plus the projection sub-tile needed later; the final
            # tile is exp-light (j=0) so the tail stays PE-bound
            sched = [(0, 0, 2), (0, 1, 3), (0, 2, 4), (0, 3, 5),
                     (1, 1, 6), (1, 2, 7), (1, 3, None), (1, 0, None)]
            pending = None
            for b, j, fsub in sched:
                q0 = b * T + j * 512
                oT0 = sb.tile([128, 512], bf16, tag="oT0", bufs=2)
                oT1 = sb.tile([128, 512], bf16, tag="oT1", bufs=2)
                fq = None
                for h in range(HPC):
                    qbuf = q01 if h < 2 else q23
                    base = (h % 2) * 64
                    oT = oT0 if h < 2 else oT1
                    pav = ps.tile([128, 512], f32, tag="av", bufs=2, name="pav")
                    nkb = 4 * j + 4
                    for pr in range(nkb // 2):
                        psc = ps.tile([128, 1024], f32, tag="sc", bufs=2, name="psc")
                        es = sb.tile([128, 1024], bf16, tag="es", bufs=3, name="es")
                        kb0, kb1 = 2 * pr, 2 * pr + 1
                        di0, di1 = kb0 - 4 * j, kb1 - 4 * j
                        c00 = 128 * di0 if di0 > 0 else 0
                        c01 = 128 * di1 if di1 > 0 else 0
                        hi = 1024 - c01
                        k0 = b * T + kb0 * 128
                        k1 = b * T + kb1 * 128
                        nc.tensor.matmul(
                            psc[:, c00:512],
                            kk[base:base + 64, k0:k0 + 128],
                            qbuf[base:base + 64, q0 + c00:q0 + 512],
                            start=True, stop=True)
                        nc.tensor.matmul(
                            psc[:, 512:hi],
                            kk[base:base + 64, k1:k1 + 128],
                            qbuf[base:base + 64, q0 + c01:q0 + 512],
                            start=True, stop=True)
                        nc.scalar.activation(es[:, c00:hi], psc[:, c00:hi],
                                             AF.Exp, scale=0.125)
                        if di0 >= 0:
                            nc.vector.tensor_mul(es[:, c00:c00 + 128],
                                                 es[:, c00:c00 + 128], tri_sb)
                            nc.vector.tensor_mul(es[:, 512:640],
                                                 es[:, 512:640], tri_sb)
                        nc.tensor.matmul(
                            pav[0:65, c00:512],
                            vnat[:, b * KB + kb0, :],
                            es[:, c00:512],
                            start=(kb0 == 0), stop=False)
                        nc.tensor.matmul(
                            pav[0:65, c01:512],
                            vnat[:, b * KB + kb1, :],
                            es[:, 512:hi],
                            start=False, stop=(kb1 == nkb - 1))
                    rrow = sb.tile([1, 512], f32, tag="rr", bufs=2, name="rr")
                    nc.vector.tensor_copy(rrow, pav[64:65, :])
                    rec = sb.tile([1, 512], f32, tag="rec", bufs=2, name="rec")
                    nc.vector.reciprocal_approx_fast(out=rec, in_=rrow)
                    rb = sb.tile([64, 512], f32, tag="rb", bufs=2, name="rb")
                    nc.gpsimd.partition_broadcast(rb, rec)
                    nc.vector.tensor_mul(oT[base:base + 64, :], pav[0:64, :], rb)
                    if pending is not None and not ((b, j) == sched[-1][:2]
                                                    and h == 3):
                        # (reserve the very last pending slice: it covers the
                        # final head's normalize-chain latency after the loop)
                        oproj_slice(pending, h, cast_scalar=(j <= 1))
                    emit_bulk(2)
                    if fsub is not None:
                        if h == 0:
                            fq = proj_q2(fsub)
                        elif h == 1:
                            rope_q(fsub, 0, fq[0])
                            rope_q(fsub, 1, fq[1])
                        elif h == 2:
                            fq = proj_kv(fsub)
                        else:
                            rope_kv(fsub, *fq)
                if (b, j) == sched[-1][:2]:
                    oproj_slice(pending, 3, cast_scalar=True)
                pending = (oT0, oT1, q0)
            for st in range(4):
                oproj_slice(pending, st, cast_scalar=True)
            if DEBUG:
                nc.sync.dma_start(out=dbg_q01[:], in_=q01[:])
                nc.sync.dma_start(out=dbg_q23[:], in_=q23[:])
                nc.sync.dma_start(out=dbg_kk[:], in_=kk[:])
                nc.sync.dma_start(
                    out=dbg_vnat[:].rearrange("p (b c) -> p b c", b=N // 128),
                    in_=vnat[:])

    nc.compile()
    _nc_cache[0] = nc
    return nc


def prep_inputs(x, wq, wk, wv, wo):
    x = np.asarray(x, np.float32)
    wq = np.asarray(wq, np.float32)
    wk = np.asarray(wk, np.float32)
    wv = np.asarray(wv, np.float32)
    wo = np.asarray(wo, np.float32)

    xt = np.ascontiguousarray(x.reshape(N, D).T.astype(ml_dtypes.bfloat16))  # [D, N]

    # de-interleave RoPE pairs inside each head's 64 columns: re/im partners
    # land 16 partitions apart so the on-device swap is one stream_shuffle
    # (which can only permute within 32-partition quadrants)
    deint = np.concatenate([
        np.arange(0, 32, 2), np.arange(1, 32, 2),      # re_0..15, im_0..15
        np.arange(32, 64, 2), np.arange(33, 64, 2),    # re_16..31, im_16..31
    ])
    qperm = (np.arange(H)[:, None] * HD + deint[None, :]).reshape(-1)
    kperm = (np.arange(HKV)[:, None] * HD + deint[None, :]).reshape(-1)
    wq_p = wq[:, qperm]
    wk_p = wk[:, kperm]

    # rope tables in the matching row order
    inv = 1.0 / (ROPE_THETA ** (np.arange(0, HD, 2, dtype=np.float64) / HD))
    tpos = np.arange(T, dtype=np.float64)
    ang = np.outer(tpos, inv)                                        # [T, 32]
    cosv = np.cos(ang).astype(np.float32).T                          # [32, T]
    sinv = np.sin(ang).astype(np.float32).T
    fidx = np.concatenate([np.arange(16), np.arange(16),
                           np.arange(16, 32), np.arange(16, 32)])
    sgn = np.concatenate([-np.ones(16), np.ones(16),
                          -np.ones(16), np.ones(16)]).astype(np.float32)
    cos_half = cosv[fidx]                                            # [64, T]
    sin_half = sinv[fidx] * sgn[:, None]
    cs = np.stack([
        np.tile(np.tile(cos_half, (2, 1)), (1, B)),
        np.tile(np.tile(sin_half, (2, 1)), (1, B)),
    ]).astype(ml_dtypes.bfloat16)                                    # [2, 128, N]

    p = np.arange(128)[:, None]
    c = np.arange(128)[None, :]
    tri = (p <= c).astype(ml_dtypes.bfloat16)                        # [128, 128]

    ones = np.ones((128, 32), ml_dtypes.bfloat16)
    ident = np.eye(64, dtype=np.float32)

    in_maps = []
    for core in range(NCORES):
        wq_c = wq_p[:, core * DQC:(core + 1) * DQC]
        wk_c = wk_p[:, core * HD:(core + 1) * HD]
        wv_c = wv[:, core * HD:(core + 1) * HD]
        wqkv = np.ascontiguousarray(
            np.concatenate([wq_c, wk_c, wv_c], axis=1).astype(ml_dtypes.bfloat16))
        wo_c = np.ascontiguousarray(
            wo[core * DQC:(core + 1) * DQC, :].astype(ml_dtypes.bfloat16))
        in_maps.append({
            "xt": xt, "wqkv": wqkv, "wo": wo_c, "cs": cs,
            "tri": tri, "ones": ones, "ident": ident,
        })
    return in_maps


def kernel(x, wq, wk, wv, wo):
    nc = build()
    in_maps = prep_inputs(x, wq, wk, wv, wo)
    res = run_bass_kernel_spmd(nc, in_maps, list(range(NCORES)))
    acc = np.zeros((N, D), np.float64)
    for core in range(NCORES):
        acc += res.results[core]["out"].astype(np.float32)
    return acc.astype(np.float32).reshape(B, T, D)


# revision 50
# speedup vs baseline: 1.0230x; 1.0069x over previous
"""Llama GQA attention (B=2, T=2048, D=2048, 32 heads / 8 KV heads, hd=64) on
8 Trainium2 NeuronCores.

Strategy: tensor-parallel over heads. Each core owns 4 q-heads + 1 kv-head:
wq/wk/wv output-dim sharded, wo input-dim sharded; each core emits a partial
[4096, 2048] o-proj output and the host sums the 8 partials.

Device-side layout tricks:
  - Fully fused single-phase pipeline: projections, RoPE, attention and
    o-proj live in one tile-pool scope so the Tile scheduler can interleave
    them freely.  Projection matmuls for the sub-tile needed two attention
    tiles later act as PE filler inside scalar-bound attention stretches;
    the previous tile's o-proj fills the rest.  This keeps the PE busy
    end-to-end (the PE p-state drops to 1.2 GHz after any idle gap and needs
    3us of continuous work to re-reach 2.4 GHz).
  - x is shipped pre-transposed (xt [2048, 4096]); q/k/v projections run as
    out[d, t] = w.T @ xt with no on-device transposes of x.
  - RoPE pair de-interleave is folded into a host-side permutation of the
    wq/wk columns, arranged so the re/im partner sits 16 partitions away:
    the on-device partner fetch is a single DVE stream_shuffle (no DMA) and
    the rotation itself is all-bf16 SBUF math on the DVE.
  - v is transposed to natural layout with f32 PE transposes packed 4-to-a-
    PSUM-tile, drained by one strided scalar copy.
  - Softmax runs on transposed scores (scoresT[k, q]); the denominator comes
    free from a ones-column appended to v; no max-subtraction is needed
    (inputs are scaled so exp cannot overflow).
  - Causality at block granularity; within diagonal-band k-block pairs the
    second block's live columns are packed immediately after the first's so
    a single Exp covers exactly the causal area (no wasted ACT columns).
  - PSUM budget (8 banks): scores 2x[128,1024] (4), AV accum 2x[128,512]
    (2), o-proj/projection shared 2x[128,512] (2).
  - All DMAs ride the SP (sync) queue: DMA triggers on the Activation queue
    hit fabric flow-control waits that block the exps behind them.  Bulk x
    is emitted in small rate-limited batches between compute groups so no
    flow-control wait ever sits ahead of latency-critical transfers, and
    constants are interleaved into the first x group by need time.
  - The exp-light (j=0) tile runs last so the tail (final o-proj + reserved
    o-proj slice) stays PE-bound instead of draining behind the exp stream.
"""
import sys

for _p in ("/opt/trn_rl_repo", "/root/.axon_site", "/root/.axon_site/_ro/trn_rl_repo",
           "/root/.axon_site/_ro/pypackages"):
    if _p not in sys.path:
        sys.path.append(_p)

import numpy as np
import ml_dtypes

import concourse.bass as bass
import concourse.mybir as mybir
import concourse.tile as tile
from concourse import bacc
from concourse.bass_utils import run_bass_kernel_spmd

f32 = mybir.dt.float32
bf16 = mybir.dt.bfloat16
AF = mybir.ActivationFunctionType

B, T, D = 2, 2048, 2048
H, HKV, HD = 32, 8, 64
NCORES = 8
HPC = H // NCORES            # q heads per core (4)
DQC = HPC * HD               # 256 q channels per core
N = B * T                    # 4096 flattened tokens
KC = D // 128                # 16 contraction chunks for projections
NT = N // 512                # 8 token sub-tiles of 512 for projections
QT = T // 512                # 4 q-tiles of 512 per batch
KB = T // 128                # 16 k-blocks of 128 per batch
ROPE_THETA = 10000.0

_nc_cache = [None]


def build():
    if _nc_cache[0] is not None:
        return _nc_cache[0]
    nc = bacc.Bacc()
    xt = nc.declare_dram_parameter("xt", [D, N], bf16, isOutput=False)
    wqkv = nc.declare_dram_parameter("wqkv", [D, DQC + 2 * HD], bf16, isOutput=False)
    wo = nc.declare_dram_parameter("wo", [DQC, D], bf16, isOutput=False)
    cs = nc.declare_dram_parameter("cs", [2, 128, N], bf16, isOutput=False)
    tri = nc.declare_dram_parameter("tri", [128, 128], bf16, isOutput=False)
    ident = nc.declare_dram_parameter("ident", [64, 64], f32, isOutput=False)
    ones = nc.declare_dram_parameter("ones", [128, 32], bf16, isOutput=False)
    out = nc.declare_dram_parameter("out", [N, D], bf16, isOutput=True)
    DEBUG = bool(__import__("os").environ.get("KDBG"))
    if DEBUG:
        dbg_q01 = nc.declare_dram_parameter("dbg_q01", [128, N], bf16, isOutput=True)
        dbg_q23 = nc.declare_dram_parameter("dbg_q23", [128, N], bf16, isOutput=True)
        dbg_kk = nc.declare_dram_parameter("dbg_kk", [128, N], bf16, isOutput=True)
        dbg_vnat = nc.declare_dram_parameter("dbg_vnat", [128, (N // 128) * 65], bf16,
                                             isOutput=True)

    with tile.TileContext(nc) as tc:
        with tc.tile_pool(name="pers", bufs=1) as pers, \
             tc.tile_pool(name="sb", bufs=1) as sb, \
             tc.tile_pool(name="ps", bufs=1, space="PSUM") as ps:
            wqkv_sb = pers.tile([128, KC, 384], bf16)
            wo0 = pers.tile([128, D], bf16)
            wo1 = pers.tile([128, D], bf16)
            tri_sb = pers.tile([128, 128], bf16)
            ident_f = pers.tile([64, 64], f32)
            q01 = pers.tile([128, N], bf16)      # heads 0,1 qT
            q23 = pers.tile([128, N], bf16)      # heads 2,3 qT
            kk = pers.tile([128, N], bf16)       # kT duplicated into both halves
            vnat = pers.tile([128, N // 128, 65], bf16)  # v natural + ones col
            cos_f = pers.tile([128, N], bf16)
            sin_f = pers.tile([128, N], bf16)

            # ---- input DMAs ----
            # sync (SP) queue: weights interleaved with group-0 x chunks so the
            # first projection matmul can start as soon as chunk 0 lands.
            # Everything rides the sync (SP) queue: keeping the Scalar engine
            # queue free of DMA triggers matters — fabric flow-control waits
            # there would block the exps/copies queued behind them.
            # Constants are interleaved into the wqkv+g0 stream by need time.
            xg = [[None] * KC for _ in range(4)]
            for c in range(KC):
                nc.sync.dma_start(out=wqkv_sb[:, c, :],
                                  in_=wqkv[c * 128:(c + 1) * 128, :])
                t_ = sb.tile([128, 1024], bf16, tag="xg", bufs=48, name=f"xg0_{c}")
                nc.sync.dma_start(out=t_, in_=xt[c * 128:(c + 1) * 128, 0:1024])
                xg[0][c] = t_
                if c == 2:
                    nc.sync.dma_start(out=tri_sb, in_=tri[:])
                    nc.sync.dma_start(out=ident_f, in_=ident[:])
                    nc.sync.dma_start(out=vnat[:, :, 64:65],
                                      in_=ones[:].unsqueeze(2))
                elif c == 7:
                    # only batch 0's half is needed before sub-4's rope; the
                    # rest loads via the bulk list once the fabric has slack
                    nc.sync.dma_start(out=cos_f[:, 0:2048], in_=cs[0, :, 0:2048])
                elif c == 8:
                    nc.sync.dma_start(out=sin_f[:, 0:2048], in_=cs[1, :, 0:2048])

            # Remaining bulk (x groups 1-3, wo) rides the sync queue in small
            # rate-limited batches sprinkled through the compute stream, so no
            # flow-control wait ever sits ahead of latency-critical work.
            bulk = []
            for g in range(1, 4):
                for c in range(KC):
                    t_ = sb.tile([128, 1024], bf16, tag="xg", bufs=48,
                                 name=f"xg{g}_{c}")
                    xg[g][c] = t_
                    bulk.append((t_, xt[c * 128:(c + 1) * 128,
                                        g * 1024:(g + 1) * 1024]))
                if g == 1:
                    bulk.append((wo0, wo[0:128, :]))
                    bulk.append((wo1, wo[128:256, :]))
                    bulk.append((cos_f[:, 2048:], cs[0, :, 2048:]))
                    bulk.append((sin_f[:, 2048:], cs[1, :, 2048:]))

            def emit_bulk(k):
                for _ in range(k):
                    if bulk:
                        dst, src = bulk.pop(0)
                        nc.sync.dma_start(out=dst, in_=src)

            # ---- projection + RoPE helpers ----
            def proj_q2(n):
                # both q head-pairs chunk-major: consumption tracks x-chunk
                # arrival so the PE queue never stalls deep inside a burst
                g, s0 = n // 2, (n % 2) * 512
                pqa = ps.tile([128, 512], f32, tag="op", bufs=2, name=f"pq0_{n}")
                pqb = ps.tile([128, 512], f32, tag="op", bufs=2, name=f"pq1_{n}")
                for c in range(KC):
                    st, sp = (c == 0), (c == KC - 1)
                    nc.tensor.matmul(pqa, wqkv_sb[:, c, 0:128],
                                     xg[g][c][:, s0:s0 + 512], start=st, stop=sp)
                    nc.tensor.matmul(pqb, wqkv_sb[:, c, 128:256],
                                     xg[g][c][:, s0:s0 + 512], start=st, stop=sp)
                qca = sb.tile([128, 512], bf16, tag="qc", bufs=3, name="qc")
                nc.scalar.copy(qca, pqa)
                qcb = sb.tile([128, 512], bf16, tag="qc", bufs=3, name="qc")
                nc.scalar.copy(qcb, pqb)
                return qca, qcb

            # rope pair partner: a +-16 partition shift within each 32-quadrant
            # (host channel layout arranged so this is one DVE stream_shuffle)
            SWAP16 = list(range(16, 32)) + list(range(16))

            def rope_q(n, part, qc):
                t0 = n * 512
                qbuf = q01 if part == 0 else q23
                qs = sb.tile([128, 512], bf16, tag="qs", bufs=3, name="qs")
                nc.vector.stream_shuffle(qs, qc, SWAP16)
                t1 = sb.tile([128, 512], bf16, tag="t1", bufs=3, name="t1")
                t2 = sb.tile([128, 512], bf16, tag="t2", bufs=3, name="t2")
                nc.vector.tensor_mul(t1, qc, cos_f[:, t0:t0 + 512])
                nc.vector.tensor_mul(t2, qs, sin_f[:, t0:t0 + 512])
                nc.vector.tensor_add(qbuf[:, t0:t0 + 512], t1, t2)

            def proj_kv(n, tag="op"):
                g, s0 = n // 2, (n % 2) * 512
                pkv = ps.tile([128, 512], f32, tag=tag, bufs=2, name=f"pkv_{n}")
                for c in range(KC):
                    nc.tensor.matmul(pkv, wqkv_sb[:, c, 256:384],
                                     xg[g][c][:, s0:s0 + 512],
                                     start=(c == 0), stop=(c == KC - 1))
                kc_ = sb.tile([64, 512], bf16, tag="qc", bufs=3, name="kc")
                nc.scalar.copy(kc_, pkv[0:64, :])
                vts = sb.tile([64, 512], f32, tag="vts", bufs=2, name="vts")
                nc.scalar.copy(vts, pkv[64:128, :])
                return kc_, vts

            def rope_kv(n, kc_, vts):
                t0 = n * 512
                ks = sb.tile([64, 512], bf16, tag="qs", bufs=3, name="ks")
                nc.vector.stream_shuffle(ks, kc_, SWAP16)
                t1k = sb.tile([64, 512], bf16, tag="t1", bufs=3, name="t1k")
                t2k = sb.tile([64, 512], bf16, tag="t2", bufs=3, name="t2k")
                nc.vector.tensor_mul(t1k, kc_, cos_f[0:64, t0:t0 + 512])
                nc.vector.tensor_mul(t2k, ks, sin_f[0:64, t0:t0 + 512])
                nc.vector.tensor_add(kk[0:64, t0:t0 + 512], t1k, t2k)
                nc.sync.dma_start(out=kk[64:128, t0:t0 + 512],
                                  in_=kk[0:64, t0:t0 + 512])
                # v natural layout: 4 PE transposes packed into one op-tag
                # PSUM tile, drained by a single strided scalar copy.
                trp = ps.tile([128, 512], f32, tag="op", bufs=2, name="trp")
                for s4 in range(4):
                    nc.tensor.transpose(trp[:, s4 * 64:(s4 + 1) * 64],
                                        vts[:, s4 * 128:(s4 + 1) * 128], ident_f)
                nc.scalar.copy(
                    vnat[:, n * 4:n * 4 + 4, 0:64],
                    trp[:, 0:256].rearrange("p (b v) -> p b v", b=4))

            def oproj_slice(pend, st, cast_scalar=False):
                # one quarter (token sub-block st) of a tile's o-proj
                poT0, poT1, pq0 = pend
                tk = pq0 + st * 128
                for dn2 in range(2):
                    osb = sb.tile([128, 1024], bf16, tag="osb", bufs=3, name="osb")
                    for kq in range(2):
                        dn = dn2 * 2 + kq
                        pop = ps.tile([128, 512], f32, tag="op", bufs=2, name="pop")
                        nc.tensor.matmul(pop, poT0[:, st * 128:(st + 1) * 128],
                                         wo0[:, dn * 512:(dn + 1) * 512],
                                         start=True, stop=False)
                        nc.tensor.matmul(pop, poT1[:, st * 128:(st + 1) * 128],
                                         wo1[:, dn * 512:(dn + 1) * 512],
                                         start=False, stop=True)
                        if cast_scalar and kq == 1:
                            # exp-light stretches: drain half the pops on the
                            # Scalar engine so DVE isn't the serial bottleneck
                            nc.scalar.copy(osb[:, kq * 512:(kq + 1) * 512], pop)
                        else:
                            nc.vector.tensor_copy(osb[:, kq * 512:(kq + 1) * 512],
                                                  pop)
                    nc.sync.dma_start(out=out[tk:tk + 128, dn2 * 1024:(dn2 + 1) * 1024],
                                      in_=osb)

            # ---- prologue: sub-tiles 0 and 1, chunk-major across 6 PSUM
            # accumulators so ~6 matmuls become ready per arriving x chunk
            # (the PE would otherwise sit half idle during the g0 stream).
            pq01_ = [ps.tile([128, 512], f32, tag="op", bufs=2, name=f"pq0_{n}")
                     for n in range(2)]
            pq23_ = [ps.tile([128, 1024], f32, tag="sc", bufs=2, name=f"pq1_{n}")
                     for n in range(2)]
            pkv_ = [ps.tile([128, 512], f32, tag="av", bufs=2, name=f"pkv_{n}")
                    for n in range(2)]
            for c in range(KC):
                st, sp = (c == 0), (c == KC - 1)
                for n in range(2):
                    s0 = n * 512
                    nc.tensor.matmul(pq01_[n], wqkv_sb[:, c, 0:128],
                                     xg[0][c][:, s0:s0 + 512], start=st, stop=sp)
                    nc.tensor.matmul(pq23_[n][:, 0:512], wqkv_sb[:, c, 128:256],
                                     xg[0][c][:, s0:s0 + 512], start=st, stop=sp)
                    nc.tensor.matmul(pkv_[n], wqkv_sb[:, c, 256:384],
                                     xg[0][c][:, s0:s0 + 512], start=st, stop=sp)
            for n in range(2):
                qc = sb.tile([128, 512], bf16, tag="qc", bufs=3, name="qc")
                nc.scalar.copy(qc, pq01_[n])
                rope_q(n, 0, qc)
                emit_bulk(6)
                qc = sb.tile([128, 512], bf16, tag="qc", bufs=3, name="qc")
                nc.scalar.copy(qc, pq23_[n][:, 0:512])
                rope_q(n, 1, qc)
                emit_bulk(6)
                kc_ = sb.tile([64, 512], bf16, tag="qc", bufs=3, name="kc")
                nc.scalar.copy(kc_, pkv_[n][0:64, :])
                vts = sb.tile([64, 512], f32, tag="vts", bufs=2, name="vts")
                nc.scalar.copy(vts, pkv_[n][64:128, :])
                rope_kv(n, kc_, vts)
                emit_bulk(6)

            # ---- fused attention + o-proj + interleaved projections ----
            # tile (b, j) plus the projection sub-tile needed later; the final
            # tile is exp-light (j=0) so the tail stays PE-bound
            sched = [(0, 0, 2), (0, 1, 3), (0, 2, 4), (0, 3, 5),
                     (1, 1, 6), (1, 2, 7), (1, 3, None), (1, 0, None)]
            pending = None
            for b, j, fsub in sched:
                q0 = b * T + j * 512
                oT0 = sb.tile([128, 512], bf16, tag="oT0", bufs=2)
                oT1 = sb.tile([128, 512], bf16, tag="oT1", bufs=2)
                fq = None
                for h in range(HPC):
                    qbuf = q01 if h < 2 else q23
                    base = (h % 2) * 64
                    oT = oT0 if h < 2 else oT1
                    pav = ps.tile([128, 512], f32, tag="av", bufs=2, name="pav")
                    nkb = 4 * j + 4
                    for pr in range(nkb // 2):
                        psc = ps.tile([128, 1024], f32, tag="sc", bufs=2, name="psc")
                        es = sb.tile([128, 1024], bf16, tag="es", bufs=3, name="es")
                        kb0, kb1 = 2 * pr, 2 * pr + 1
                        di0, di1 = kb0 - 4 * j, kb1 - 4 * j
                        c00 = 128 * di0 if di0 > 0 else 0
                        c01 = 128 * di1 if di1 > 0 else 0
                        hi = 1024 - c01
                        k0 = b * T + kb0 * 128
                        k1 = b * T + kb1 * 128
                        nc.tensor.matmul(
                            psc[:, c00:512],
                            kk[base:base + 64, k0:k0 + 128],
                            qbuf[base:base + 64, q0 + c00:q0 + 512],
                            start=True, stop=True)
                        nc.tensor.matmul(
                            psc[:, 512:hi],
                            kk[base:base + 64, k1:k1 + 128],
                            qbuf[base:base + 64, q0 + c01:q0 + 512],
                            start=True, stop=True)
                        nc.scalar.activation(es[:, c00:hi], psc[:, c00:hi],
                                             AF.Exp, scale=0.125)
                        if di0 >= 0:
                            nc.vector.tensor_mul(es[:, c00:c00 + 128],
                                                 es[:, c00:c00 + 128], tri_sb)
                            nc.vector.tensor_mul(es[:, 512:640],
                                                 es[:, 512:640], tri_sb)
                        nc.tensor.matmul(
                            pav[0:65, c00:512],
                            vnat[:, b * KB + kb0, :],
                            es[:, c00:512],
                            start=(kb0 == 0), stop=False)
                        nc.tensor.matmul(
                            pav[0:65, c01:512],
                            vnat[:, b * KB + kb1, :],
                            es[:, 512:hi],
                            start=False, stop=(kb1 == nkb - 1))
                    rrow = sb.tile([1, 512], f32, tag="rr", bufs=2, name="rr")
                    nc.vector.tensor_copy(rrow, pav[64:65, :])
                    rec = sb.tile([1, 512], f32, tag="rec", bufs=2, name="rec")
                    nc.vector.reciprocal_approx_fast(out=rec, in_=rrow)
                    rb = sb.tile([64, 512], f32, tag="rb", bufs=2, name="rb")
                    nc.gpsimd.partition_broadcast(rb, rec)
                    nc.vector.tensor_mul(oT[base:base + 64, :], pav[0:64, :], rb)
                    if pending is not None and not ((b, j) == sched[-1][:2]
                                                    and h == 3):
                        # (reserve the very last pending slice: it covers the
                        # final head's normalize-chain latency after the loop)
                        oproj_slice(pending, h, cast_scalar=(j <= 1))
                    emit_bulk(2)
                    if fsub is not None:
                        if h == 0:
                            fq = proj_q2(fsub)
                        elif h == 1:
                            rope_q(fsub, 0, fq[0])
                            rope_q(fsub, 1, fq[1])
                        elif h == 2:
                            fq = proj_kv(fsub)
                        else:
                            rope_kv(fsub, *fq)
                if (b, j) == sched[-1][:2]:
                    oproj_slice(pending, 3, cast_scalar=True)
                pending = (oT0, oT1, q0)
            for st in range(4):
                oproj_slice(pending, st, cast_scalar=True)
            if DEBUG:
                nc.sync.dma_start(out=dbg_q01[:], in_=q01[:])
                nc.sync.dma_start(out=dbg_q23[:], in_=q23[:])
                nc.sync.dma_start(out=dbg_kk[:], in_=kk[:])
                nc.sync.dma_start(
                    out=dbg_vnat[:].rearrange("p (b c) -> p b c", b=N // 128),
                    in_=vnat[:])

    nc.compile()
    _nc_cache[0] = nc
    return nc


def prep_inputs(x, wq, wk, wv, wo):
    x = np.asarray(x, np.float32)
    wq = np.asarray(wq, np.float32)
    wk = np.asarray(wk, np.float32)
    wv = np.asarray(wv, np.float32)
    wo = np.asarray(wo, np.float32)

    xt = np.ascontiguousarray(x.reshape(N, D).T.astype(ml_dtypes.bfloat16))  # [D, N]

    # de-interleave RoPE pairs inside each head's 64 columns: re/im partners
    # land 16 partitions apart so the on-device swap is one stream_shuffle
    # (which can only permute within 32-partition quadrants)
    deint = np.concatenate([
        np.arange(0, 32, 2), np.arange(1, 32, 2),      # re_0..15, im_0..15
        np.arange(32, 64, 2), np.arange(33, 64, 2),    # re_16..31, im_16..31
    ])
    qperm = (np.arange(H)[:, None] * HD + deint[None, :]).reshape(-1)
    kperm = (np.arange(HKV)[:, None] * HD + deint[None, :]).reshape(-1)
    wq_p = wq[:, qperm]
    wk_p = wk[:, kperm]

    # rope tables in the matching row order
    inv = 1.0 / (ROPE_THETA ** (np.arange(0, HD, 2, dtype=np.float64) / HD))
    tpos = np.arange(T, dtype=np.float64)
    ang = np.outer(tpos, inv)                                        # [T, 32]
    cosv = np.cos(ang).astype(np.float32).T                          # [32, T]
    sinv = np.sin(ang).astype(np.float32).T
    fidx = np.concatenate([np.arange(16), np.arange(16),
                           np.arange(16, 32), np.arange(16, 32)])
    sgn = np.concatenate([-np.ones(16), np.ones(16),
                          -np.ones(16), np.ones(16)]).astype(np.float32)
    cos_half = cosv[fidx]                                            # [64, T]
    sin_half = sinv[fidx] * sgn[:, None]
    cs = np.stack([
        np.tile(np.tile(cos_half, (2, 1)), (1, B)),
        np.tile(np.tile(sin_half, (2, 1)), (1, B)),
    ]).astype(ml_dtypes.bfloat16)                                    # [2, 128, N]

    p = np.arange(128)[:, None]
    c = np.arange(128)[None, :]
    tri = (p <= c).astype(ml_dtypes.bfloat16)                        # [128, 128]

    ones = np.ones((128, 32), ml_dtypes.bfloat16)
    ident = np.eye(64, dtype=np.float32)

    in_maps = []
    for core in range(NCORES):
        wq_c = wq_p[:, core * DQC:(core + 1) * DQC]
        wk_c = wk_p[:, core * HD:(core + 1) * HD]
        wv_c = wv[:, core * HD:(core + 1) * HD]
        wqkv = np.ascontiguousarray(
            np.concatenate([wq_c, wk_c, wv_c], axis=1).astype(ml_dtypes.bfloat16))
        wo_c = np.ascontiguousarray(
            wo[core * DQC:(core + 1) * DQC, :].astype(ml_dtypes.bfloat16))
        in_maps.append({
            "xt": xt, "wqkv": wqkv, "wo": wo_c, "cs": cs,
            "tri": tri, "ones": ones, "ident": ident,
        })
    return in_maps


def kernel(x, wq, wk, wv, wo):
    nc = build()
    in_maps = prep_inputs(x, wq, wk, wv, wo)
    res = run_bass_kernel_spmd(nc, in_maps, list(range(NCORES)))
    acc = np.zeros((N, D), np.float64)
    for core in range(NCORES):
        acc += res.results[core]["out"].astype(np.float32)
    return acc.astype(np.float32).reshape(B, T, D)
